# revision 13
# baseline (speedup 1.0000x reference)
"""Trainium2 Bass kernel for per-token outer-product softmax attention.

Reference computation (per token t of 1600, H=256):
    k = tanh(x W0 + b0);  q = tanh(x W1 + b1)
    scores[i,j] = k[i]*q[j];  attn = softmax_j(scores);  out = attn @ x

Key algebra: k,q are tanh outputs so k[i]*q[j] in (-1,1). exp(s) on
[-1,1] is approximated by P(s) = sum_d c_d s^d with coefficients
optimized directly for end-to-end output error; P(k_i q_j) =
sum_d c_d k_i^d q_j^d is SEPARABLE, so softmax num/den become per-token
moments:
    num_i = aN0 + sum_d (c_d sum_j q_j^d x_j) k_i^d
    den_i = aD0 + sum_d (c_d sum_j q_j^d)     k_i^d
and the 256x256 scores tensor is never materialized. D=2 suffices
(end-to-end rel-L2 ~1.3e-2 < 2e-2 tolerance).

Engine plan (per 128-token block, 2 blocks/core, tokens padded to 256):
  PE:   psQ/psK matmuls (x^T stationary, W moving) + aN0 = c0*sum_j x
        via a c0-column matmul.
  Act:  ONE wide tanh over [128,2,256] PSUM -> QK tile (Q|K).
  Pool: QK2 = QK*QK (wide), V1 = Q*X, V2 = V1*Q, uN/uD adds, final
        uN*rD multiply (f32 out).
  DVE:  4x-mode tensor_scalar ops only (127ns each): moment
        accumulations (s1,s2 from Q/Q2; m1,m2 from V1/V2 with c_d
        folded into the scalar), chain terms U2 = aN2*k^2 + aN0,
        U1 = aN1*k, T2 = aD2*k^2 + aD0, T1 = aD1*k, and the custom
        fast reciprocal (f32).

All I/O is merged: one DMA each for x (token-major), x^T, W, out.
Sharding: pure data parallel over tokens, 200 tokens/core x 8 cores
(padded to 256 with zeros; pad lanes compute harmless garbage).
"""

import numpy as np
from contextlib import ExitStack

import concourse.bass as bass
import concourse.bacc as bacc
import concourse.tile as tile
from concourse import mybir
from concourse.bass_utils import run_bass_kernel_spmd

F32 = mybir.dt.float32
F16 = mybir.dt.float16
AF = mybir.ActivationFunctionType
OP = mybir.AluOpType

B, S, M, H = 4, 10, 40, 256
T = B * S * M            # 1600 tokens
NCORES = 8
TC = T // NCORES         # 200 tokens per core
NB = 2                   # blocks of 128 (padded)
TP = 128 * NB            # padded tokens per core

# exp(t) ~ c0 + c1 t + c2 t^2 on [-1,1]; coefficients optimized for
# end-to-end rel-L2 of the full attention output (not poly minimax).
COEF = [0.98718266, 1.05076565, 0.50084856]
D = 2

CFG = {
    "x_dma": "sync",
    "xt_dma": "gpsimd",
    "w_dma": "sync",
    "out_dma": "sync",
    "out2_dma": "scalar",
    "chain": "ts_pool",   # stt | ts_pool
    "s2": "stt",          # stt | qk2
    "warm_pe": 8,         # dummy matmuls to ramp PE pstate
}


def build_kernel(reps: int = 1, with_bias: bool = True) -> bass.Bass:
    c0, c1, c2 = (float(c) for c in COEF)
    aD0 = c0 * float(H)

    nc = bacc.Bacc("TRN2", target_bir_lowering=False, debug=False)
    # xs[p, b, :] = x[token b*128+p, :]; xst[p, g, t] = x[t, g*128+p]
    xs = nc.declare_dram_parameter("xs", [128, NB, H], F16, isOutput=False)
    xst = nc.declare_dram_parameter("xst", [128, NB, TP], F16, isOutput=False)
    # wcat[:, 0:2, :] = W1 halves (queries), [:, 2:4, :] = W0 halves (keys)
    wcat = nc.declare_dram_parameter("wcat", [128, 4, H], F16, isOutput=False)
    if with_bias:
        bq = nc.declare_dram_parameter("bq", [1, H], F16, isOutput=False)
        bk = nc.declare_dram_parameter("bk", [1, H], F16, isOutput=False)
    out = nc.declare_dram_parameter("out", [128, NB, H], F32, isOutput=True)

    with tile.TileContext(nc) as tc, ExitStack() as ctx:
        consts = ctx.enter_context(tc.tile_pool(name="consts", bufs=2))
        io = ctx.enter_context(tc.tile_pool(name="io", bufs=3))
        work = ctx.enter_context(tc.tile_pool(name="work", bufs=3))
        mom = ctx.enter_context(tc.tile_pool(name="mom", bufs=2))
        scrp = ctx.enter_context(tc.tile_pool(name="scrp", bufs=8))
        psP = ctx.enter_context(tc.tile_pool(name="psP", bufs=3, space="PSUM"))

        x_eng = getattr(nc, CFG["x_dma"])
        xt_eng = getattr(nc, CFG["xt_dma"])
        w_eng = getattr(nc, CFG["w_dma"])
        out_eng = getattr(nc, CFG["out_dma"])
        out2_eng = getattr(nc, CFG["out2_dma"])

        if with_bias:
            ones1 = consts.tile([1, 128], F16)
            nc.gpsimd.memset(ones1, 1.0)

        # ---- one-time setup: weights, biases, PE warmup
        W = consts.tile([128, 4, H], F16)
        w_eng.dma_start(out=W[:, :, :], in_=wcat[:, :, :])
        if with_bias:
            bqT = consts.tile([1, H], F16)
            w_eng.dma_start(out=bqT[:, :], in_=bq[:, :])
            bkT = consts.tile([1, H], F16)
            w_eng.dma_start(out=bkT[:, :], in_=bk[:, :])
        if CFG["warm_pe"]:
            wj = consts.tile([128, H], F16)
            nc.gpsimd.memset(wj, 0.0)
            psW = ctx.enter_context(
                tc.tile_pool(name="psW", bufs=1, space="PSUM")
            )
            pw = psW.tile([1, H], F32, tag="warm")
            for _wi in range(CFG["warm_pe"]):
                nc.tensor.matmul(
                    pw[:, :], wj[:, 0:1], wj[:, :],
                    start=(_wi == 0), stop=(_wi == CFG["warm_pe"] - 1),
                )

        def head(b, XT, X):
            # psQK[:, 0, :] = x@W1 (+b1), psQK[:, 1, :] = x@W0 (+b0)
            ps = psP.tile([128, 2, H], F32, tag=f"ps{b}")
            t0 = b * 128
            for side, woff in ((0, 0), (1, 2)):
                if with_bias:
                    bt = bqT if side == 0 else bkT
                    nc.tensor.matmul(
                        ps[:, side, :], ones1[:, :], bt[:, :],
                        start=True, stop=False,
                    )
                nc.tensor.matmul(
                    ps[:, side, :], XT[:, 0, t0 : t0 + 128], W[:, woff, :],
                    start=not with_bias, stop=False,
                )
                nc.tensor.matmul(
                    ps[:, side, :], XT[:, 1, t0 : t0 + 128], W[:, woff + 1, :],
                    start=False, stop=True,
                )
            MOh = mom.tile([128, 5], F32, tag=f"MO{b}")
            ja = scrp.tile([128, H], F16, tag="scr", name=f"ja{b}")
            nc.vector.tensor_scalar(
                out=ja[:, :], in0=X[:, b, :], scalar1=c0, scalar2=0.0,
                op0=OP.mult, op1=OP.add, accum_out=MOh[:, 4:5],   # aN0
            )
            return ps, MOh

        def mid(b, st, X):
            ps, MOh = st
            Xb = X[:, b, :]
            # one wide tanh: QK[:, 0, :] = Q, QK[:, 1, :] = K
            QK = work.tile([128, 2, H], F16, tag=f"QK{b}")
            nc.scalar.activation(QK[:, :, :], ps[:, :, :], AF.Tanh)
            Q = QK[:, 0, :]
            K = QK[:, 1, :]
            if CFG["s2"] == "qk2":
                K2t = work.tile([128, 2, H], F16, tag=f"K2{b}")
                nc.gpsimd.tensor_mul(K2t[:, :, :], QK[:, :, :], QK[:, :, :])
                Q2 = K2t[:, 0, :]
                K2 = K2t[:, 1, :]
            else:
                K2t = work.tile([128, H], F16, tag=f"K2{b}")
                nc.gpsimd.tensor_mul(K2t[:, :], K, K)
                K2 = K2t[:, :]
            # raw products on Pool
            V1 = work.tile([128, H], F16, tag=f"V1{b}")
            nc.gpsimd.tensor_mul(V1[:, :], Q, Xb)
            V2 = work.tile([128, H], F16, tag=f"V2{b}")
            nc.gpsimd.tensor_mul(V2[:, :], V1[:, :], Q)
            # moment accumulations on DVE (4x tensor_scalar, coef folded)
            MO = MOh
            js = []
            for _ji in range(4):
                jt = scrp.tile([128, H], F16, tag="scr", name=f"js{b}_{_ji}")
                js.append(jt)
            nc.vector.tensor_scalar(
                out=js[0][:, :], in0=Q, scalar1=c1, scalar2=0.0,
                op0=OP.mult, op1=OP.add, accum_out=MO[:, 0:1],   # aD1
            )
            if CFG["s2"] == "qk2":
                nc.vector.tensor_scalar(
                    out=js[1][:, :], in0=Q2, scalar1=c2, scalar2=0.0,
                    op0=OP.mult, op1=OP.add, accum_out=MO[:, 1:2],  # aD2
                )
            else:
                nc.vector.scalar_tensor_tensor(
                    out=js[1][:, :], in0=Q, scalar=c2, in1=Q,
                    op0=OP.mult, op1=OP.mult, accum_out=MO[:, 1:2],  # aD2
                )
            nc.vector.tensor_scalar(
                out=js[2][:, :], in0=V1[:, :], scalar1=c1, scalar2=0.0,
                op0=OP.mult, op1=OP.add, accum_out=MO[:, 2:3],   # aN1
            )
            nc.vector.tensor_scalar(
                out=js[3][:, :], in0=V2[:, :], scalar1=c2, scalar2=0.0,
                op0=OP.mult, op1=OP.add, accum_out=MO[:, 3:4],   # aN2
            )
            return QK, K2t, MO

        def tail(b, st, O):
            QK, K2t, MO = st
            K = QK[:, 1, :]
            K2 = K2t[:, 1, :] if CFG["s2"] == "qk2" else K2t[:, :]
            # uN = aN0 + aN1 k + aN2 k^2
            U1 = work.tile([128, H], F16, tag=f"U1{b}")
            nc.vector.tensor_scalar(
                out=U1[:, :], in0=K, scalar1=MO[:, 2:3], scalar2=MO[:, 4:5],
                op0=OP.mult, op1=OP.add,
            )
            # uD = aD0 + aD1 k + aD2 k^2 (f32 for the custom reciprocal)
            T1 = work.tile([128, H], F32, tag=f"T1{b}")
            nc.vector.tensor_scalar(
                out=T1[:, :], in0=K, scalar1=MO[:, 0:1], scalar2=aD0,
                op0=OP.mult, op1=OP.add,
            )
            if CFG["chain"] == "stt":
                uN = work.tile([128, H], F16, tag=f"uN{b}")
                nc.vector.scalar_tensor_tensor(
                    out=uN[:, :], in0=K2, scalar=MO[:, 3:4], in1=U1[:, :],
                    op0=OP.mult, op1=OP.add,
                )
                uD = work.tile([128, H], F32, tag=f"uD{b}")
                nc.vector.scalar_tensor_tensor(
                    out=uD[:, :], in0=K2, scalar=MO[:, 1:2], in1=T1[:, :],
                    op0=OP.mult, op1=OP.add,
                )
            else:
                U2 = work.tile([128, H], F16, tag=f"U2{b}")
                nc.vector.tensor_scalar(
                    out=U2[:, :], in0=K2, scalar1=MO[:, 3:4], scalar2=None,
                    op0=OP.mult,
                )
                T2 = work.tile([128, H], F32, tag=f"T2{b}")
                nc.vector.tensor_scalar(
                    out=T2[:, :], in0=K2, scalar1=MO[:, 1:2], scalar2=None,
                    op0=OP.mult,
                )
                uN = work.tile([128, H], F16, tag=f"uN{b}")
                nc.gpsimd.tensor_add(uN[:, :], U1[:, :], U2[:, :])
                uD = work.tile([128, H], F32, tag=f"uD{b}")
                nc.gpsimd.tensor_add(uD[:, :], T1[:, :], T2[:, :])
            rD = work.tile([128, H], F32, tag=f"rD{b}")
            nc.vector.reciprocal_approx_fast(rD[:, :], uD[:, :])
            nc.gpsimd.tensor_mul(O[:, b, :], uN[:, :], rD[:, :])
            eng = out_eng if b == 0 else out2_eng
            eng.dma_start(out=out[:, b, :], in_=O[:, b, :])

        def body():
            XT = io.tile([128, NB, TP], F16, tag="XT")
            xt_eng.dma_start(out=XT[:, :, :], in_=xst[:, :, :])
            X = io.tile([128, NB, H], F16, tag="X")
            x_eng.dma_start(out=X[:, :, :], in_=xs[:, :, :])
            O = io.tile([128, NB, H], F32, tag="O")
            sts = [head(b, XT, X) for b in range(NB)]
            sts = [mid(b, sts[b], X) for b in range(NB)]
            for b in range(NB):
                tail(b, sts[b], O)

        if reps == 1:
            body()
        else:
            with tc.For_i(0, reps, 1):
                body()

    nc.compile()
    return nc


_NCS = {}


def _get_nc(with_bias: bool = True):
    if with_bias not in _NCS:
        _NCS[with_bias] = build_kernel(with_bias=with_bias)
    return _NCS[with_bias]


def _make_in_maps(x, W0, b0, W1, b1):
    xf = np.asarray(x, np.float32).reshape(T, H).astype(np.float16)
    W0h = np.asarray(W0, np.float32).astype(np.float16)
    W1h = np.asarray(W1, np.float32).astype(np.float16)
    wcat = np.ascontiguousarray(
        np.stack(
            [W1h[:128, :], W1h[128:, :], W0h[:128, :], W0h[128:, :]], axis=1
        )
    )  # [128, 4, 256]
    with_bias = bool(
        np.any(np.asarray(b0, np.float32)) or np.any(np.asarray(b1, np.float32))
    )
    maps = []
    for c in range(NCORES):
        sh = np.zeros((TP, H), np.float16)
        sh[:TC] = xf[c * TC : (c + 1) * TC]
        xs = np.ascontiguousarray(sh.reshape(NB, 128, H).transpose(1, 0, 2))
        xst = np.ascontiguousarray(sh.reshape(TP, 2, 128).transpose(2, 1, 0))
        m = {"xs": xs, "xst": xst, "wcat": wcat}
        if with_bias:
            m["bq"] = np.asarray(b1, np.float32).astype(np.float16).reshape(1, H)
            m["bk"] = np.asarray(b0, np.float32).astype(np.float16).reshape(1, H)
        maps.append(m)
    return maps


def _ensure_axon():
    try:
        import jax
        if not any(d.platform == "axon" for d in jax.devices()):
            jax.config.update("jax_platforms", "axon,cpu")
    except Exception:
        pass


def _run(x, W0, b0, W1, b1, trace=False, **kw):
    _ensure_axon()
    with_bias = bool(
        np.any(np.asarray(b0, np.float32)) or np.any(np.asarray(b1, np.float32))
    )
    res = run_bass_kernel_spmd(
        _get_nc(with_bias), _make_in_maps(x, W0, b0, W1, b1),
        list(range(NCORES)), trace=trace, **kw,
    )
    outs = []
    for c in range(NCORES):
        o = res.results[c]["out"]  # [128, NB, H]
        outs.append(o.transpose(1, 0, 2).reshape(TP, H)[:TC])
    full = np.concatenate(outs, axis=0).reshape(B, S, M, H).astype(np.float32)
    return full, res


def kernel(x, W0, b0, W1, b1):
    full, _ = _run(x, W0, b0, W1, b1, trace=False)
    return full


# revision 14
# speedup vs baseline: 1.0362x; 1.0362x over previous
"""Trainium2 Bass kernel for per-token outer-product softmax attention.

Reference computation (per token t of 1600, H=256):
    k = tanh(x W0 + b0);  q = tanh(x W1 + b1)
    scores[i,j] = k[i]*q[j];  attn = softmax_j(scores);  out = attn @ x

Key algebra: k,q are tanh outputs so k[i]*q[j] in (-1,1). exp(s) on
[-1,1] is approximated by P(s) = sum_d c_d s^d with coefficients
optimized directly for end-to-end output error; P(k_i q_j) =
sum_d c_d k_i^d q_j^d is SEPARABLE, so softmax num/den become per-token
moments:
    num_i = aN0 + sum_d (c_d sum_j q_j^d x_j) k_i^d
    den_i = aD0 + sum_d (c_d sum_j q_j^d)     k_i^d
and the 256x256 scores tensor is never materialized. D=2 suffices
(end-to-end rel-L2 ~1.3e-2 < 2e-2 tolerance).

Engine plan (per 128-token block, 2 blocks/core, tokens padded to 256):
  PE:   psQ/psK matmuls (x^T stationary, W moving) + aN0 = c0*sum_j x
        via a c0-column matmul.
  Act:  ONE wide tanh over [128,2,256] PSUM -> QK tile (Q|K).
  Pool: QK2 = QK*QK (wide), V1 = Q*X, V2 = V1*Q, uN/uD adds, final
        uN*rD multiply (f32 out).
  DVE:  4x-mode tensor_scalar ops only (127ns each): moment
        accumulations (s1,s2 from Q/Q2; m1,m2 from V1/V2 with c_d
        folded into the scalar), chain terms U2 = aN2*k^2 + aN0,
        U1 = aN1*k, T2 = aD2*k^2 + aD0, T1 = aD1*k, and the custom
        fast reciprocal (f32).

All I/O is merged: one DMA each for x (token-major), x^T, W, out.
Sharding: pure data parallel over tokens, 200 tokens/core x 8 cores
(padded to 256 with zeros; pad lanes compute harmless garbage).
"""

import numpy as np
from contextlib import ExitStack

import concourse.bass as bass
import concourse.bacc as bacc
import concourse.tile as tile
from concourse import mybir
from concourse.bass_utils import run_bass_kernel_spmd

F32 = mybir.dt.float32
F16 = mybir.dt.float16
AF = mybir.ActivationFunctionType
OP = mybir.AluOpType

B, S, M, H = 4, 10, 40, 256
T = B * S * M            # 1600 tokens
NCORES = 8
TC = T // NCORES         # 200 tokens per core
NB = 2                   # blocks of 128 (padded)
TP = 128 * NB            # padded tokens per core

# exp(t) ~ c0 + c1 t + c2 t^2 on [-1,1]; coefficients optimized for
# end-to-end rel-L2 of the full attention output (not poly minimax).
COEF = [0.98718266, 1.05076565, 0.50084856]
D = 2

CFG = {
    "x_dma": "sync",
    "xt_dma": "gpsimd",
    "w_dma": "sync",
    "out_dma": "sync",
    "out2_dma": "scalar",
    "chain": "ts_pool",   # stt | ts_pool
    "s2": "stt",          # stt | qk2
    "warm_pe": 8,         # dummy matmuls to ramp PE pstate
}


def build_kernel(reps: int = 1, with_bias: bool = True) -> bass.Bass:
    c0, c1, c2 = (float(c) for c in COEF)
    aD0 = c0 * float(H)

    nc = bacc.Bacc("TRN2", target_bir_lowering=False, debug=False)
    # xs[p, b, :] = x[token b*128+p, :]; xst[p, g, t] = x[t, g*128+p]
    xs = nc.declare_dram_parameter("xs", [128, NB, H], F16, isOutput=False)
    xst = nc.declare_dram_parameter("xst", [128, NB, TP], F16, isOutput=False)
    # wcat[:, 0:2, :] = W1 halves (queries), [:, 2:4, :] = W0 halves (keys)
    wcat = nc.declare_dram_parameter("wcat", [128, 4, H], F16, isOutput=False)
    if with_bias:
        bq = nc.declare_dram_parameter("bq", [1, H], F16, isOutput=False)
        bk = nc.declare_dram_parameter("bk", [1, H], F16, isOutput=False)
    out = nc.declare_dram_parameter("out", [128, NB, H], F32, isOutput=True)

    with tile.TileContext(nc) as tc, ExitStack() as ctx:
        consts = ctx.enter_context(tc.tile_pool(name="consts", bufs=2))
        io = ctx.enter_context(tc.tile_pool(name="io", bufs=3))
        work = ctx.enter_context(tc.tile_pool(name="work", bufs=3))
        mom = ctx.enter_context(tc.tile_pool(name="mom", bufs=2))
        scrp = ctx.enter_context(tc.tile_pool(name="scrp", bufs=8))
        psP = ctx.enter_context(tc.tile_pool(name="psP", bufs=3, space="PSUM"))

        x_eng = getattr(nc, CFG["x_dma"])
        xt_eng = getattr(nc, CFG["xt_dma"])
        w_eng = getattr(nc, CFG["w_dma"])
        out_eng = getattr(nc, CFG["out_dma"])
        out2_eng = getattr(nc, CFG["out2_dma"])

        if with_bias:
            ones1 = consts.tile([1, 128], F16)
            nc.gpsimd.memset(ones1, 1.0)

        if CFG["warm_pe"]:
            wj = consts.tile([128, H], F16)
            nc.gpsimd.memset(wj, 0.0)
            psW = ctx.enter_context(
                tc.tile_pool(name="psW", bufs=1, space="PSUM")
            )

        def head(b, XT, X, W, bias):
            # psQK[:, 0, :] = x@W1 (+b1), psQK[:, 1, :] = x@W0 (+b0)
            ps = psP.tile([128, 2, H], F32, tag=f"ps{b}")
            t0 = b * 128
            for side, woff in ((0, 0), (1, 2)):
                if with_bias:
                    bt = bias[side]
                    nc.tensor.matmul(
                        ps[:, side, :], ones1[:, :], bt[:, :],
                        start=True, stop=False,
                    )
                nc.tensor.matmul(
                    ps[:, side, :], XT[:, 0, t0 : t0 + 128], W[:, woff, :],
                    start=not with_bias, stop=False,
                )
                nc.tensor.matmul(
                    ps[:, side, :], XT[:, 1, t0 : t0 + 128], W[:, woff + 1, :],
                    start=False, stop=True,
                )
            MOh = mom.tile([128, 5], F32, tag=f"MO{b}")
            ja = scrp.tile([128, H], F16, tag="scr", name=f"ja{b}")
            nc.vector.tensor_scalar(
                out=ja[:, :], in0=X[:, b, :], scalar1=c0, scalar2=0.0,
                op0=OP.mult, op1=OP.add, accum_out=MOh[:, 4:5],   # aN0
            )
            return ps, MOh

        def mid(b, st, X):
            ps, MOh = st
            Xb = X[:, b, :]
            # one wide tanh: QK[:, 0, :] = Q, QK[:, 1, :] = K
            QK = work.tile([128, 2, H], F16, tag=f"QK{b}")
            nc.scalar.activation(QK[:, :, :], ps[:, :, :], AF.Tanh)
            Q = QK[:, 0, :]
            K = QK[:, 1, :]
            if CFG["s2"] == "qk2":
                K2t = work.tile([128, 2, H], F16, tag=f"K2{b}")
                nc.gpsimd.tensor_mul(K2t[:, :, :], QK[:, :, :], QK[:, :, :])
                Q2 = K2t[:, 0, :]
                K2 = K2t[:, 1, :]
            else:
                K2t = work.tile([128, H], F16, tag=f"K2{b}")
                nc.gpsimd.tensor_mul(K2t[:, :], K, K)
                K2 = K2t[:, :]
            # raw products on Pool
            V1 = work.tile([128, H], F16, tag=f"V1{b}")
            nc.gpsimd.tensor_mul(V1[:, :], Q, Xb)
            V2 = work.tile([128, H], F16, tag=f"V2{b}")
            nc.gpsimd.tensor_mul(V2[:, :], V1[:, :], Q)
            # moment accumulations on DVE (4x tensor_scalar, coef folded)
            MO = MOh
            js = []
            for _ji in range(4):
                jt = scrp.tile([128, H], F16, tag="scr", name=f"js{b}_{_ji}")
                js.append(jt)
            nc.vector.tensor_scalar(
                out=js[0][:, :], in0=Q, scalar1=c1, scalar2=0.0,
                op0=OP.mult, op1=OP.add, accum_out=MO[:, 0:1],   # aD1
            )
            if CFG["s2"] == "qk2":
                nc.vector.tensor_scalar(
                    out=js[1][:, :], in0=Q2, scalar1=c2, scalar2=0.0,
                    op0=OP.mult, op1=OP.add, accum_out=MO[:, 1:2],  # aD2
                )
            else:
                nc.vector.scalar_tensor_tensor(
                    out=js[1][:, :], in0=Q, scalar=c2, in1=Q,
                    op0=OP.mult, op1=OP.mult, accum_out=MO[:, 1:2],  # aD2
                )
            nc.vector.tensor_scalar(
                out=js[2][:, :], in0=V1[:, :], scalar1=c1, scalar2=0.0,
                op0=OP.mult, op1=OP.add, accum_out=MO[:, 2:3],   # aN1
            )
            nc.vector.tensor_scalar(
                out=js[3][:, :], in0=V2[:, :], scalar1=c2, scalar2=0.0,
                op0=OP.mult, op1=OP.add, accum_out=MO[:, 3:4],   # aN2
            )
            return QK, K2t, MO

        def tail(b, st, O):
            QK, K2t, MO = st
            K = QK[:, 1, :]
            K2 = K2t[:, 1, :] if CFG["s2"] == "qk2" else K2t[:, :]
            # uN = aN0 + aN1 k + aN2 k^2
            U1 = work.tile([128, H], F16, tag=f"U1{b}")
            nc.vector.tensor_scalar(
                out=U1[:, :], in0=K, scalar1=MO[:, 2:3], scalar2=MO[:, 4:5],
                op0=OP.mult, op1=OP.add,
            )
            # uD = aD0 + aD1 k + aD2 k^2 (f32 for the custom reciprocal)
            T1 = work.tile([128, H], F32, tag=f"T1{b}")
            nc.vector.tensor_scalar(
                out=T1[:, :], in0=K, scalar1=MO[:, 0:1], scalar2=aD0,
                op0=OP.mult, op1=OP.add,
            )
            if CFG["chain"] == "stt":
                uN = work.tile([128, H], F16, tag=f"uN{b}")
                nc.vector.scalar_tensor_tensor(
                    out=uN[:, :], in0=K2, scalar=MO[:, 3:4], in1=U1[:, :],
                    op0=OP.mult, op1=OP.add,
                )
                uD = work.tile([128, H], F32, tag=f"uD{b}")
                nc.vector.scalar_tensor_tensor(
                    out=uD[:, :], in0=K2, scalar=MO[:, 1:2], in1=T1[:, :],
                    op0=OP.mult, op1=OP.add,
                )
            else:
                U2 = work.tile([128, H], F16, tag=f"U2{b}")
                nc.vector.tensor_scalar(
                    out=U2[:, :], in0=K2, scalar1=MO[:, 3:4], scalar2=None,
                    op0=OP.mult,
                )
                T2 = work.tile([128, H], F32, tag=f"T2{b}")
                nc.vector.tensor_scalar(
                    out=T2[:, :], in0=K2, scalar1=MO[:, 1:2], scalar2=None,
                    op0=OP.mult,
                )
                uN = work.tile([128, H], F16, tag=f"uN{b}")
                nc.gpsimd.tensor_add(uN[:, :], U1[:, :], U2[:, :])
                uD = work.tile([128, H], F32, tag=f"uD{b}")
                nc.gpsimd.tensor_add(uD[:, :], T1[:, :], T2[:, :])
            rD = work.tile([128, H], F32, tag=f"rD{b}")
            nc.vector.reciprocal_approx_fast(rD[:, :], uD[:, :])
            nc.gpsimd.tensor_mul(O[:, b, :], uN[:, :], rD[:, :])
            eng = out_eng if b == 0 else out2_eng
            eng.dma_start(out=out[:, b, :], in_=O[:, b, :])

        def body():
            if CFG["warm_pe"]:
                pw = psW.tile([1, H], F32, tag="warm")
                for _wi in range(CFG["warm_pe"]):
                    nc.tensor.matmul(
                        pw[:, :], wj[:, 0:1], wj[:, :],
                        start=(_wi == 0), stop=(_wi == CFG["warm_pe"] - 1),
                    )
            W = consts.tile([128, 4, H], F16)
            w_eng.dma_start(out=W[:, :, :], in_=wcat[:, :, :])
            if with_bias:
                bqT = consts.tile([1, H], F16)
                w_eng.dma_start(out=bqT[:, :], in_=bq[:, :])
                bkT = consts.tile([1, H], F16)
                w_eng.dma_start(out=bkT[:, :], in_=bk[:, :])
            XT = io.tile([128, NB, TP], F16, tag="XT")
            xt_eng.dma_start(out=XT[:, :, :], in_=xst[:, :, :])
            X = io.tile([128, NB, H], F16, tag="X")
            x_eng.dma_start(out=X[:, :, :], in_=xs[:, :, :])
            O = io.tile([128, NB, H], F32, tag="O")
            bias = (bqT, bkT) if with_bias else None
            sts = [head(b, XT, X, W, bias) for b in range(NB)]
            sts = [mid(b, sts[b], X) for b in range(NB)]
            for b in range(NB):
                tail(b, sts[b], O)

        if reps == 1:
            body()
        else:
            with tc.For_i(0, reps, 1):
                body()

    nc.compile()
    return nc


_NCS = {}


def _get_nc(with_bias: bool = True):
    if with_bias not in _NCS:
        _NCS[with_bias] = build_kernel(with_bias=with_bias)
    return _NCS[with_bias]


def _make_in_maps(x, W0, b0, W1, b1):
    xf = np.asarray(x, np.float32).reshape(T, H).astype(np.float16)
    W0h = np.asarray(W0, np.float32).astype(np.float16)
    W1h = np.asarray(W1, np.float32).astype(np.float16)
    wcat = np.ascontiguousarray(
        np.stack(
            [W1h[:128, :], W1h[128:, :], W0h[:128, :], W0h[128:, :]], axis=1
        )
    )  # [128, 4, 256]
    with_bias = bool(
        np.any(np.asarray(b0, np.float32)) or np.any(np.asarray(b1, np.float32))
    )
    maps = []
    for c in range(NCORES):
        sh = np.zeros((TP, H), np.float16)
        sh[:TC] = xf[c * TC : (c + 1) * TC]
        xs = np.ascontiguousarray(sh.reshape(NB, 128, H).transpose(1, 0, 2))
        xst = np.ascontiguousarray(sh.reshape(TP, 2, 128).transpose(2, 1, 0))
        m = {"xs": xs, "xst": xst, "wcat": wcat}
        if with_bias:
            m["bq"] = np.asarray(b1, np.float32).astype(np.float16).reshape(1, H)
            m["bk"] = np.asarray(b0, np.float32).astype(np.float16).reshape(1, H)
        maps.append(m)
    return maps


def _ensure_axon():
    try:
        import jax
        if not any(d.platform == "axon" for d in jax.devices()):
            jax.config.update("jax_platforms", "axon,cpu")
    except Exception:
        pass


def _run(x, W0, b0, W1, b1, trace=False, **kw):
    _ensure_axon()
    with_bias = bool(
        np.any(np.asarray(b0, np.float32)) or np.any(np.asarray(b1, np.float32))
    )
    res = run_bass_kernel_spmd(
        _get_nc(with_bias), _make_in_maps(x, W0, b0, W1, b1),
        list(range(NCORES)), trace=trace, **kw,
    )
    outs = []
    for c in range(NCORES):
        o = res.results[c]["out"]  # [128, NB, H]
        outs.append(o.transpose(1, 0, 2).reshape(TP, H)[:TC])
    full = np.concatenate(outs, axis=0).reshape(B, S, M, H).astype(np.float32)
    return full, res


def kernel(x, W0, b0, W1, b1):
    full, _ = _run(x, W0, b0, W1, b1, trace=False)
    return full
